# revision 12
# baseline (speedup 1.0000x reference)
"""Trainium2 Bass kernel for the Chebyshev spectral layer.

Computation (per reference):
  x_cheb = DCT-I(x)[..., :512];  om = einsum('bix,iox->box', x_cheb, w)
  out = IDCT-I(pad(om))  ==  om @ M2  with M2[k,n] = cos(pi*k*n/2047)*c2[k]

Sharding: data-parallel over batch. 8 cores, 8 batches each. The DCT
matrices and weights are replicated (host-precomputed constants).

Per-core dataflow (all fp32, matmuls in float32r):
  T1  PE-transpose x [bi,n] -> XT [n,bi] in 128x128 blocks
  S1  x_cheb psum [bi,k] = sum_j XT_j.T @ M1T_j   (4 bi-chunks x 16 n-chunks)
      evacuate with partition-shifted copies -> XC2 [i=64, b=8, k=512]
  S2  per-mode k: psum[o, b] = Wc[:,:,k].T @ XC2[:,:,k]  (512 matmuls)
      stacked in psum free dim -> OM [o=64, kc=4, kl*8+b]
  T2  PE-transpose per (b,kc): OM[o, kl] -> OMT [kl=128, kc, b, o]
  S3  out psum [128=(b2,o), n] = sum_ch OMT[:,ch,2bp:2bp+2,:].T @ M2P[:,ch,:]
"""
import numpy as np

import concourse.bass as bass
import concourse.tile as tile
from concourse import mybir
from concourse.bass_utils import run_bass_kernel_spmd
from concourse.vector_clock import ScopedClock

F32 = mybir.dt.float32
F32R = mybir.dt.float32r


class SplitDrainTC(tile.TileContext):
    """Walrus in this container rejects >1 sync-wait per instruction. Split
    extra waits onto same-engine NoOps emitted immediately before the
    instruction (identical semantics: conjunction of sem waits in program
    order)."""

    MAX_WAITS = 1

    def _add_instruction(self, inst):
        si = inst.sync_info
        if si is not None and si.on_wait and len(si.on_wait) > self.MAX_WAITS:
            waits = list(si.on_wait)
            si.on_wait = waits[: self.MAX_WAITS]
            for w in waits[self.MAX_WAITS:]:
                nop = mybir.InstNoOp(
                    name=self.nc.get_next_instruction_name(), ins=[], outs=[]
                )
                nop.engine = inst.engine
                nop.sync_info = mybir.SyncInfo(on_wait=[w], on_update=[])
                super()._add_instruction(nop)
        super()._add_instruction(inst)

    def _drain_and_barrier(self, tick_clock, wait_clock):
        drain_inst = self.nc.sync.drain()
        wait_clock.add_sem_waits(
            drain_inst.ins, ScopedClock({None: tick_clock.global_clock})
        )
        si = drain_inst.ins.sync_info
        waits = list(si.on_wait or []) if si else []
        if len(waits) > 1:
            si.on_wait = waits[:1]
            for w in waits[1:]:
                d2 = self.nc.sync.drain()
                d2.ins.sync_info = mybir.SyncInfo(on_wait=[w], on_update=[])
        self.nc.all_engine_barrier()
        popped = self.nc._tile_sem_poison_stack.pop()
        assert popped is self._sem_poison
        self.nc.clear_and_free_semaphores(list(self.sems.allocated().values()))
        self.nc.all_engine_barrier()

B, IC, OC, NG, MD = 64, 64, 64, 2048, 512
NCORES = 8
BPC = B // NCORES          # 8 batches per core
P = 128

_CACHE = {}


def _constants():
    if "m1t" in _CACHE:
        return _CACHE["m1t"], _CACHE["m2p"]
    n = np.arange(NG)
    k = np.arange(MD)
    # C[n, k] = cos(pi*n*k/(NG-1)) computed in f64 then cast
    C = np.cos(np.pi * np.outer(n, k) / (NG - 1))
    c = np.full(NG, 2.0); c[0] = 1.0; c[-1] = 1.0
    c2 = np.full(MD, 2.0); c2[0] = 1.0
    M1T = (C * c[:, None]).astype(np.float32)              # [n, k]
    M2 = (C.T * c2[:, None]).astype(np.float32)            # [k, n]
    # SBUF layouts: m1t [128, 16, 512]  (partition p = n % 128, j = n // 128)
    m1t = np.ascontiguousarray(M1T.reshape(16, 128, MD).transpose(1, 0, 2))
    # m2p [128, 4, 2048] (partition p = k % 128, ch = k // 128)
    m2p = np.ascontiguousarray(M2.reshape(4, 128, NG).transpose(1, 0, 2))
    _CACHE["m1t"], _CACHE["m2p"] = m1t, m2p
    return m1t, m2p


def _build_nc(reps: int = 1):
    nc = bass.Bass("TRN2", target_bir_lowering=False)
    x_s = nc.dram_tensor("x_s", [BPC * IC, NG], F32R, kind="ExternalInput")
    wt = nc.dram_tensor("wt", [4, 64, 64 * 128], F32R, kind="ExternalInput")
    m1t = nc.dram_tensor("m1t", [P, 16 * MD], F32R, kind="ExternalInput")
    m2p = nc.dram_tensor("m2p", [P, 4 * NG], F32R, kind="ExternalInput")
    idm = nc.dram_tensor("idm", [P, P], F32R, kind="ExternalInput")
    o_s = nc.dram_tensor("o_s", [BPC * OC, NG], F32, kind="ExternalOutput")

    x_ap = x_s.ap()
    wt_ap = wt.ap()
    m1t_ap = m1t.ap().rearrange("p (j k) -> p j k", j=16)
    m2p_ap = m2p.ap().rearrange("p (c n) -> p c n", c=4)
    o_ap = o_s.ap()

    with SplitDrainTC(nc) as tc:
        with (
            tc.tile_pool(name="const", bufs=1) as const,
            tc.tile_pool(name="big", bufs=1) as big,
        ):
            ident = const.tile([P, P], F32R)
            nc.sync.dma_start(ident[:], idm.ap())

            xc2 = big.tile([64, BPC, MD], F32R)       # [i, b, k]
            om = big.tile([64, 4, 8 * P], F32R)       # [o, kc, kl*8+b]
            omt = big.tile([P, 4, BPC, 64], F32R)     # [kl, kc, b, o]

            if reps == 1:
                _phase12(nc, tc, x_ap, wt_ap, m1t_ap, m2p_ap, o_ap,
                         ident, xc2, om, omt)
            else:
                with tc.For_i(0, reps, 1):
                    _phase12(nc, tc, x_ap, wt_ap, m1t_ap, m2p_ap, o_ap,
                             ident, xc2, om, omt)
    return nc


def _phase12(nc, tc, x_ap, wt_ap, m1t_ap, m2p_ap, o_ap, ident, xc2, om, omt):
        with (
            tc.tile_pool(name="xb", bufs=1) as xb_pool,
            tc.tile_pool(name="m1", bufs=2) as m1_pool,
            tc.tile_pool(name="xt", bufs=3) as xt_pool,
            tc.tile_pool(name="ps_s1", bufs=1, space="PSUM") as ps_s1,
            tc.tile_pool(name="ps_xt", bufs=2, space="PSUM") as ps_xt,
        ):
            # ---------------- T1 + S1 ----------------
            s1ps = []
            xbs = []
            for ch in range(4):
                xb = xb_pool.tile([P, NG], F32R, tag=f"xb{ch}")
                nc.sync.dma_start(xb[:], x_ap[ch * P:(ch + 1) * P, :])
                xbs.append(xb)
                s1ps.append(ps_s1.tile([P, MD], F32, tag=f"s1_{ch}", name=f"s1ps{ch}"))
            for j in range(16):
                m1j = m1_pool.tile([P, MD], F32R, tag="m1")
                nc.sync.dma_start(m1j[:], m1t_ap[:, j, :])
                for ch in range(4):
                    xb = xbs[ch]
                    tps = ps_xt.tile([P, P], F32R, tag="xtps")
                    nc.tensor.transpose(tps[:], xb[:, j * P:(j + 1) * P], ident[:])
                    xt = xt_pool.tile([P, P], F32R, tag="xt")
                    nc.any.tensor_copy(out=xt[:], in_=tps[:])
                    nc.tensor.matmul(
                        s1ps[ch][:],
                        xt[:],
                        m1j[:],
                        start=(j == 0),
                        stop=(j == 15),
                    )
            # evacuate with partition-shifted copies -> XC2 [i, b, k]
            for ch in range(4):
                nc.vector.tensor_copy(out=xc2[:, 2 * ch, :], in_=s1ps[ch][0:64, :])
                nc.vector.tensor_copy(out=xc2[:, 2 * ch + 1, :], in_=s1ps[ch][64:P, :])

        with (
            tc.tile_pool(name="wc2", bufs=2) as wc_pool,
            tc.tile_pool(name="m2b", bufs=1) as m2_pool,
            tc.tile_pool(name="osb2", bufs=3) as osb_pool,
            tc.tile_pool(name="ps_s2", bufs=2, space="PSUM") as ps_s2,
            tc.tile_pool(name="ps_t2", bufs=2, space="PSUM") as ps_t2,
            tc.tile_pool(name="ps_s3", bufs=2, space="PSUM") as ps_s3,
        ):
            # ---------------- S2 ----------------
            for kc in range(4):
                wc = wc_pool.tile([64, 64, P], F32R, tag="wc")
                nc.sync.dma_start(
                    wc[:], wt_ap[kc].rearrange("i (o l) -> i o l", l=P)
                )
                p2 = ps_s2.tile([64, 8 * P], F32, tag="s2")
                for kl in range(P):
                    k = kc * P + kl
                    nc.tensor.matmul(
                        p2[:, kl * 8:(kl + 1) * 8],
                        wc[:, :, kl],
                        xc2[:, :, k],
                        start=True,
                        stop=True,
                    )
                nc.any.tensor_copy(out=om[:, kc, :], in_=p2[:])

            # ---------------- T2 ----------------
            for b in range(BPC):
                for kc in range(4):
                    tps = ps_t2.tile([P, 64], F32R, tag="t2")
                    nc.tensor.transpose(
                        tps[:],
                        om[:, kc, b::8],
                        ident[0:64, 0:64],
                    )
                    nc.any.tensor_copy(out=omt[:, kc, b, :], in_=tps[:])

            # ---------------- S3 ----------------
            m2t = []
            for chv in range(4):
                t = m2_pool.tile([P, NG], F32R, tag=f"m2_{chv}", name=f"m2t{chv}")
                nc.sync.dma_start(t[:], m2p_ap[:, chv, :])
                m2t.append(t)
            for bp in range(4):
                for nb in range(4):
                    ps3 = ps_s3.tile([P, 512], F32, tag="s3")
                    for ch in range(4):
                        nc.tensor.matmul(
                            ps3[:],
                            omt[:, ch, 2 * bp:2 * bp + 2, :],
                            m2t[ch][:, nb * 512:(nb + 1) * 512],
                            start=(ch == 0),
                            stop=(ch == 3),
                        )
                    osb = osb_pool.tile([P, 512], F32, tag="osb")
                    nc.any.tensor_copy(out=osb[:], in_=ps3[:])
                    nc.sync.dma_start(
                        o_ap[bp * P:(bp + 1) * P, nb * 512:(nb + 1) * 512], osb[:]
                    )


def kernel(x: np.ndarray, weights: np.ndarray) -> np.ndarray:
    x = np.ascontiguousarray(np.asarray(x, dtype=np.float32))
    w = np.ascontiguousarray(np.asarray(weights, dtype=np.float32))
    m1t, m2p = _constants()
    wt = np.ascontiguousarray(
        w.reshape(IC, OC, 4, 128).transpose(2, 0, 1, 3).reshape(4, 64, 64 * 128)
    )
    m1t_f = m1t.reshape(P, 16 * MD)
    m2p_f = m2p.reshape(P, 4 * NG)

    if "nc" not in _CACHE:
        _CACHE["nc"] = _build_nc()
    nc = _CACHE["nc"]

    in_maps = []
    for c in range(NCORES):
        xs = x[c * BPC:(c + 1) * BPC].reshape(BPC * IC, NG)
        in_maps.append({
            "x_s": np.ascontiguousarray(xs),
            "wt": wt,
            "m1t": m1t_f,
            "m2p": m2p_f,
            "idm": np.eye(P, dtype=np.float32),
        })
    res = run_bass_kernel_spmd(nc, in_maps, core_ids=list(range(NCORES)), trace=False)
    out = np.empty((B, OC, NG), dtype=np.float32)
    for c in range(NCORES):
        out[c * BPC:(c + 1) * BPC] = res.results[c]["o_s"].reshape(BPC, OC, NG)
    return out
